# revision 20
# baseline (speedup 1.0000x reference)
"""Trainium2 Bass kernel for nn_DeformConv_49890340110644.

3x3 SAME conv, NCHW (8,32,512,512) x OIHW (32,32,3,3) + bias -> (8,32,512,512).

Strategy: data-parallel over batch (1 image per NeuronCore, 8 cores; no
collectives). Per core ("kpack" layout):

- Input rows are interleaved across the 4 partition groups of SBUF:
  partition 32*(row mod 4)+ci, so X[:, t, :] holds 4 consecutive rows
  across the full 128 partitions. Each row is DMA'd exactly once.
- Output rows are produced 4 at a time ("quad" starting at row Y, Y=4m):
  one PSUM bank [128, 512] holds (r, co) = out rows Y..Y+3 x 32 channels.
- Per quad and filter column kw, two accumulating matmuls:
    A: K=128 over X[:, p, :]     (input rows Y-1..Y+2), lhsT = banded
       block-Toeplitz [128,128] with W[co,ci,i-r,kw] at block (i, r)
    B: K=64  over X[0:64, p+1, :] (input rows Y+3, Y+4)
  The kw=+-1 column shifts are handled by shrinking the matmul's N range
  (PSUM per-element overwrite covers the untouched edge column); the
  kw=1 full-width matmul starts each accumulation group. Row -1 / row
  512 halos use extra weight slots with the halo K-block zeroed.
- VectorE evacuates each bank with a fused per-channel bias add; output
  DMA'd back with the inverse row interleave.

Rows are processed in blocks of 32 (8 PSUM banks per block, 16 blocks),
with double-buffered input/output tiles so DMA overlaps PE compute.
Total per core: 48 N=512 matmuls x 16 blocks = 768 matmuls.
"""

import os
import sys

sys.path.insert(0, "/opt/trn_rl_repo")

import numpy as np

N_IMG, C, H, W = 8, 32, 512, 512
RB = 32            # output rows per block
NBLK = H // RB     # 16
WP = W + 2         # padded row width
TIN = 9            # row slots per partition group

# matmul input dtype: "fp32r" (full-rate fp32, tf32-ish precision),
# "fp32" (exact, quarter-rate), "bf16" (half DMA, bf16 precision)
MM_CFG = os.environ.get("CONV_MM_DTYPE", "fp32r")
# layout: "tap" = 9 taps/row as K=32,M=32 matmuls on 16 subarrays (tile_position)
#         "kpack" = 6 K=128/64,M=128 matmuls per 4-row quad (banded weights)
LAYOUT = os.environ.get("CONV_LAYOUT", "kpack")

_cache: dict = {}


def _build(cfg: str, layout: str):
    import concourse.mybir as mybir
    import concourse.tile as tile
    from concourse import bacc

    F32 = mybir.dt.float32
    MM_DT = {
        "fp32": F32,
        "fp32r": mybir.dt.float32r,
        "bf16": mybir.dt.bfloat16,
        "fp16": mybir.dt.float16,
    }[cfg]

    nc = bacc.Bacc(None)
    x_d = nc.declare_dram_parameter("x", [C, H, W], MM_DT, isOutput=False)
    if layout == "tap":
        w_d = nc.declare_dram_parameter("w", [C, TIN, C], MM_DT, isOutput=False)
    else:
        w_d = nc.declare_dram_parameter("w", [12, 128, 128], MM_DT, isOutput=False)
    b_d = nc.declare_dram_parameter("b", [C, 1], F32, isOutput=False)
    o_d = nc.declare_dram_parameter("o", [C, H, W], F32, isOutput=True)

    with tile.TileContext(nc) as tc:
        with (
            tc.tile_pool(name="const", bufs=1) as cpool,
            tc.tile_pool(name="xin", bufs=2) as xpool,
            tc.tile_pool(name="out", bufs=2) as opool,
            tc.tile_pool(name="psum", bufs=8, space="PSUM") as ppool,
        ):
            Bsb = cpool.tile([128, 1], F32)
            if layout == "tap":
                Wsb = cpool.tile([128, TIN, C], MM_DT)
                for g in range(4):
                    nc.sync.dma_start(
                        out=Wsb[32 * g : 32 * g + 32, :, :], in_=w_d[:, :, :]
                    )
            else:
                Wsb = cpool.tile([128, 12, 128], MM_DT)
                nc.sync.dma_start(
                    out=Wsb[:, :, :], in_=w_d[:, :, :].rearrange("s k m -> k s m")
                )
            for g in range(4):
                nc.sync.dma_start(out=Bsb[32 * g : 32 * g + 32, :], in_=b_d[:, :])

            for b in range(NBLK):
                y0 = RB * b
                XW = WP if layout == "tap" else W
                xoff = 1 if layout == "tap" else 0
                X = xpool.tile([128, TIN, XW], MM_DT)

                if layout == "tap":
                    # zero the column pads (kw-shift halo)
                    nc.vector.memset(X[:, :, 0 : WP : WP - 1], 0.0)

                # load input rows r = y0-1 .. y0+32 at (g = (r-y0+1)%4, t)
                # group g holds rows y0-1+g, y0+3+g, ... (step 4)
                for g in range(4):
                    r_start = y0 - 1 + g
                    cnt = 9 if g < 2 else 8
                    t0 = 0
                    if r_start < 0:  # block 0, g=0 slot t=0 is row -1
                        if layout == "tap":
                            nc.vector.memset(X[0:32, 0, :], 0.0)
                        else:
                            # filler data; the A0 weight slots zero this block
                            nc.sync.dma_start(
                                out=X[0:32, 0, xoff : xoff + W], in_=x_d[:, 0:1, :]
                            )
                        r_start += 4
                        t0 = 1
                        cnt -= 1
                    if r_start + 4 * (cnt - 1) > H - 1:  # block 15, g=1 slot t=8 is row 512
                        if layout == "tap":
                            nc.vector.memset(X[32:64, TIN - 1, :], 0.0)
                        else:
                            # filler data; the B7 weight slots zero this block
                            nc.sync.dma_start(
                                out=X[32:64, TIN - 1, xoff : xoff + W],
                                in_=x_d[:, H - 1 : H, :],
                            )
                        cnt -= 1
                    nc.sync.dma_start(
                        out=X[32 * g : 32 * g + 32, t0 : t0 + cnt, xoff : xoff + W],
                        in_=x_d[:, r_start : r_start + 4 * (cnt - 1) + 1 : 4, :],
                    )

                O = opool.tile([128, 8, W], F32)
                if layout == "tap":
                    for p in range(8):
                        P = ppool.tile([128, W], mybir.dt.float32)
                        for qj in range(4):
                            q = 4 * p + qj
                            j = q % 4
                            for kh in range(3):
                                k = q + kh
                                g, t = k % 4, k // 4
                                for kw in range(3):
                                    tap = 3 * kh + kw
                                    nc.tensor.matmul(
                                        P[32 * j : 32 * j + 32, :],
                                        Wsb[32 * g : 32 * g + 32, tap, :],
                                        X[32 * g : 32 * g + 32, t, kw : kw + W],
                                        start=(tap == 0),
                                        stop=(tap == 8),
                                        tile_position=(32 * g, 32 * j),
                                    )
                        # evacuate bank p with fused bias add
                        nc.vector.tensor_scalar_add(O[:, p, :], P[:, :], Bsb[:, 0:1])
                else:
                    # kpack: quad p covers rows Y=y0+4p..Y+3.
                    # A-matmul: K=128 over X[:, p] (rows Y-1..Y+2)
                    # B-matmul: K=64 over X[0:64, p+1] (rows Y+3, Y+4)
                    # kw edge columns: out col range [max(0,1-kw) : min(W, W+1-kw)]
                    # reads input cols shifted by kw-1; kw=1 (full width) goes
                    # first so the accumulation group's start matmul covers the
                    # whole PSUM bank.
                    Ps = [
                        ppool.tile([128, W], mybir.dt.float32, name=f"P{b}_{pp}", tag="P")
                        for pp in range(8)
                    ]
                    for si, (var, kw) in enumerate(
                        [(0, 1), (0, 0), (0, 2), (1, 1), (1, 0), (1, 2)]
                    ):
                        o_lo = max(0, 1 - kw)
                        o_hi = min(W, W + 1 - kw)
                        i_lo = o_lo + kw - 1
                        for p in range(8):
                            if var == 0:
                                s = kw
                                if b == 0 and p == 0:
                                    s = 6 + kw  # A0: row -1 block zeroed
                                nc.tensor.matmul(
                                    Ps[p][:, o_lo:o_hi],
                                    Wsb[:, s, :],
                                    X[:, p, i_lo : i_lo + (o_hi - o_lo)],
                                    start=(si == 0),
                                    stop=False,
                                )
                            else:
                                s = 3 + kw
                                if b == NBLK - 1 and p == 7:
                                    s = 9 + kw  # B7: row 512 block zeroed
                                nc.tensor.matmul(
                                    Ps[p][:, o_lo:o_hi],
                                    Wsb[0:64, s, :],
                                    X[0:64, p + 1, i_lo : i_lo + (o_hi - o_lo)],
                                    start=False,
                                    stop=(si == 5),
                                )
                    for p in range(8):
                        nc.vector.tensor_scalar_add(O[:, p, :], Ps[p][:, :], Bsb[:, 0:1])

                for j in range(4):
                    nc.sync.dma_start(
                        out=o_d[:, y0 + j : y0 + RB : 4, :],
                        in_=O[32 * j : 32 * j + 32, :, :],
                    )

    nc.finalize()
    return nc


def _kpack_weights(weight):
    """Banded block-Toeplitz lhsT set, [12, 128, 128] = (slot, K, M).

    Slot kw (A): K partition (i,ci) holds input row Y-1+i; M (r,co) is
    output row Y+r -> block value W[co,ci,i-r,kw] when 0<=i-r<=2.
    Slot 3+kw (B): K partition (g,ci) holds row Y+3+g (g<2) -> kh=4+g-r.
    Slot 6+kw (A0): A with the i=0 block zeroed (row -1, block 0 quad 0).
    Slot 9+kw (B7): B with the g=1 block zeroed (row 512, block 15 quad 7).
    """
    wk = np.zeros((12, 128, 128), dtype=np.float32)
    wt = weight.transpose(1, 0, 2, 3)  # [ci, co, kh, kw]
    for kw in range(3):
        for i in range(4):
            for r in range(4):
                kh = i - r
                if 0 <= kh <= 2:
                    wk[kw, 32 * i : 32 * i + 32, 32 * r : 32 * r + 32] = wt[:, :, kh, kw]
        for g in range(2):
            for r in range(4):
                kh = 4 + g - r
                if 0 <= kh <= 2:
                    wk[3 + kw, 32 * g : 32 * g + 32, 32 * r : 32 * r + 32] = wt[:, :, kh, kw]
    wk[6:9] = wk[0:3]
    wk[6:9, 0:32, :] = 0.0
    wk[9:12] = wk[3:6]
    wk[9:12, 32:64, :] = 0.0
    return wk


def _prep_inputs(input, weight, bias, cfg, layout):
    """Host-side prep: per-core input maps."""
    if layout == "tap":
        wp = np.ascontiguousarray(
            weight.transpose(1, 2, 3, 0).reshape(C, TIN, C)
        )  # [ci, kh*kw, co]
    else:
        wp = _kpack_weights(weight)
    if cfg in ("bf16", "fp16"):
        import ml_dtypes

        np_dt = ml_dtypes.bfloat16 if cfg == "bf16" else np.float16
        xs = [np.ascontiguousarray(input[i]).astype(np_dt) for i in range(N_IMG)]
        wp = wp.astype(np_dt)
    else:
        xs = [np.ascontiguousarray(input[i], dtype=np.float32) for i in range(N_IMG)]
        wp = wp.astype(np.float32)
    b2 = np.ascontiguousarray(bias.reshape(C, 1), dtype=np.float32)
    return [{"x": xs[i], "w": wp, "b": b2} for i in range(N_IMG)]


def kernel(input, weight, bias, _trace=False):
    from concourse.bass_utils import run_bass_kernel_spmd

    input = np.asarray(input)
    weight = np.asarray(weight)
    bias = np.asarray(bias)

    cfg, layout = MM_CFG, LAYOUT
    if (cfg, layout) not in _cache:
        _cache[(cfg, layout)] = _build(cfg, layout)
    nc = _cache[(cfg, layout)]

    in_maps = _prep_inputs(input, weight, bias, cfg, layout)
    res = run_bass_kernel_spmd(nc, in_maps, list(range(N_IMG)), trace=_trace)
    out = np.stack([res.results[i]["o"] for i in range(N_IMG)]).astype(np.float32)
    if _trace:
        return out, res
    return out
